# revision 22
# baseline (speedup 1.0000x reference)
"""Batch-assign-probability (VQ codebook softmax) kernel for 8 Trainium2 cores.

Math: for each valid row x (D=512), over K=256 centers c_k:
    softmax_k(-||x - c_k||^2) == softmax_k(2 x.c_k - ||c_k||^2)
(the ||x||^2 term is constant over k and cancels in softmax).

v2 design (vs the 3-pass bf16 baseline; 44.1us -> ~30.6us measured):
  - fp16 single-pass main matmul: x_fp16 @ (2c)_fp16^T, full PE rate. fp16's
    10+1 mantissa bits put the logit error at ~7e-3 rms; one extra
    correction pass kills the centers-side quantization error:
        corr = x_e5m2 @ ((2c) - fp16(2c))_e5m2
    run as 2 fp8 DoubleRow matmuls (256-contraction per inst, one weight
    load each). End-to-end max abs output err 9.58e-3 vs the 2e-2 gate
    (deterministic; HW matches the numpy simulation to ~1e-5).
  - The -||c||^2 bias rides the correction pass in 3 sacrificed contraction
    rows (x8 rows 509-511 := 1.0, ctl8 rows 509-511 := 3-level e5m2 bias
    split); the residual is compensated exactly on the host. No bias matmul.
  - exp(l - rowmax) on ACT writes fp16 directly; softmax NORMALIZATION
    (exp-bias residual weight + row-sum division) happens on the HOST,
    removing the accum-read + reciprocal + broadcast-multiply chain from
    the device. Scalar engine runs only the 16 exps (+1 tail out-DMA).
  - All x traffic for a row group rides ONE byte-packed DMA (fp16 bytes
    then fp8 bytes per partition), bitcast back into typed views on SBUF:
    3.1MB/core of x instead of 4.2MB, few descriptors on the sync ring
    (each DMA_DIRECT2D costs ~650ns of issue + the ring is FIFO, so
    fewer/bigger transfers win; out-DMAs ride behind the x stream).
  - Device per 128-row tile: 4 fp16 + 2 fp8-DR matmuls -> PSUM (the
    steady-state cost is 6 LDWEIGHTS ~= 670ns/tile; the PE issue itself is
    ~540ns), DVE reduce_max(negate) per pair, ACT exp -> fp16 og, per-group
    out-DMA (last group from the scalar queue to dodge the sync tail).
  - Set USE_CCORR=False for a ~0.8us faster / err 1.51e-2 variant that
    drops the correction pass and the x8 stream.
"""

import numpy as np
import ml_dtypes

import concourse.bacc as bacc
import concourse.tile as tile
from concourse import mybir
from concourse.bass_utils import run_bass_kernel_spmd

B, T, W, C, K = 16, 2048, 512, 1, 256
VALID_T = 1024
D = W * C                       # 512
N_CORES = 8
B_PER_CORE = B // N_CORES       # 2
ROWS = B_PER_CORE * VALID_T     # 2048 rows per core
P = 128
DC = D // P                     # 4 contraction chunks
GROUPS = [128, 256, 512, 512, 512, 128]   # rows per x/out DMA group
N_WARM_MM = 4                   # dummy matmuls to lift the PE HAM clock-gate
assert sum(GROUPS) == ROWS
assert all(g % P == 0 for g in GROUPS)

# USE_CCORR=True: a 2-inst fp8 DoubleRow pass corrects the fp16(2c)
# quantization error (max err ~9.6e-3) and carries the -||c||^2 bias in 3
# sacrificed contraction rows. USE_CCORR=False: skip the correction (max err
# ~1.50e-2, still under the 2e-2 gate), drop the x8 stream (2.1MB instead of
# 3.1MB of x per core) and carry the bias in a dedicated 1-inst DoubleRow
# matmul against tiny resident constants: 5 instead of 6 weight-load-bound
# matmuls per 128-row tile. Bias residual is compensated exactly by the
# host-side softmax weight either way.
USE_CCORR = True
XB = 12 if USE_CCORR else 8     # x bytes per row element-slot per partition
CONST_B = (2048 + 1024) if USE_CCORR else (2048 + 256 + 512)

F16 = np.float16
E5M2 = ml_dtypes.float8_e5m2

_CACHE: dict = {}


def _build_bass():
    f32 = mybir.dt.float32
    f16 = mybir.dt.float16
    f8 = mybir.dt.float8e5
    u8 = mybir.dt.uint8
    nc = bacc.Bacc()
    xp = nc.declare_dram_parameter("xp", [P * XB * ROWS], u8, isOutput=False)
    cp = nc.declare_dram_parameter("cp", [P * CONST_B], u8, isOutput=False)
    out = nc.declare_dram_parameter("out", [ROWS, K], f16, isOutput=True)
    out_v = out.rearrange("(t p) k -> p t k", p=P)       # [128, 16, 256]

    with tile.TileContext(nc) as tc:
        with (
            tc.tile_pool(name="singles", bufs=1) as singles,
            tc.tile_pool(name="xpool", bufs=1) as xpool,
            tc.tile_pool(name="opool", bufs=3) as opool,
            tc.tile_pool(name="small", bufs=8) as small,
            tc.tile_pool(name="psum", bufs=7, space="PSUM") as psum,
            tc.tile_pool(name="psum_warm", bufs=1, space="PSUM") as psum_warm,
        ):
            # constants first on the sync ring: they gate the first matmul
            # together with group 0. ct and ctl ride separate DMAs (deps are
            # byte-range granular) so the first fp16 matmuls gate on ct +
            # g0's xh bytes only (~0.39MB), not the full const+g0 0.59MB.
            csb = singles.tile([P, CONST_B], u8)
            c_ap = csb[:]
            cpv = cp.rearrange("(p b) -> p b", p=P)
            nc.sync.dma_start(out=c_ap[:], in_=cpv[:])
            ct_v = c_ap[:, :2 * DC * K].bitcast(f16).rearrange(
                "p (c k) -> p c k", c=DC)
            o = 2 * DC * K
            if not USE_CCORR:
                # bias carrier: lhsT ones (rows 0-2 of slice 0), rhs bias rows
                bones_v = c_ap[:, o:o + 2 * P].bitcast(f8).rearrange(
                    "p (i r) -> p i r", i=2)
                o += 2 * P
                brhs_v = c_ap[:, o:o + 2 * K].bitcast(f8).rearrange(
                    "p (i k) -> p i k", i=2)

            xgs = []
            xoff = 0
            for g, R in enumerate(GROUPS):
                n = P * XB * R
                xg = xpool.tile([P, XB * R], u8, tag=f"xg{g}")
                xsrc = xp[xoff:xoff + n].rearrange("(p b) -> p b", p=P)
                ap = xg[:]
                # odd groups ride the scalar ring (Q10): the two HWDGE rings
                # pull concurrently across the 16 DMA engines, so delivery of
                # consecutive groups overlaps instead of serializing on Q1
                (nc.scalar if g % 2 == 1 else nc.sync).dma_start(
                    out=ap[:], in_=xsrc)
                xoff += n
                xh_v = ap[:, :8 * R].bitcast(f16).rearrange(
                    "p (c r) -> p c r", c=DC)
                x8_v = None
                if USE_CCORR:
                    x8_v = ap[:, 8 * R:].bitcast(f8).rearrange(
                        "p (j i r) -> p j i r", j=2, i=2)
                xgs.append((xh_v, x8_v))
            ctl_v = None
            if USE_CCORR:
                ctl_v = c_ap[:, 2 * DC * K:].bitcast(f8).rearrange(
                    "p (j i k) -> p j i k", j=2, i=2)

            # PE warm-up: dummy matmuls keep the PE busy through the HAM
            # activity window while the first x DMA lands.
            warm_sb = singles.tile([P, 512], f16)
            nc.gpsimd.memset(warm_sb[:], 0.0)
            warm_ps = psum_warm.tile([P, 512], f32, tag="warm")
            for _ in range(N_WARM_MM):
                nc.tensor.matmul(
                    warm_ps[:], lhsT=warm_sb[:, :P], rhs=warm_sb[:],
                    start=True, stop=True,
                )

            t0 = 0  # running 128-row tile index
            for g, R in enumerate(GROUPS):
                xh_v, x8_v = xgs[g]
                subtiles = R // P
                og = opool.tile([P, subtiles, K], f16, tag="og")
                last_g = g == len(GROUPS) - 1
                for s0 in range(0, subtiles, 2):
                    pair = min(2, subtiles - s0)
                    ps = psum.tile([P, pair, K], f32, tag="ps")
                    for j in range(pair):
                        s = s0 + j
                        rsl = slice(s * P, (s + 1) * P)
                        for c in range(DC):
                            nc.tensor.matmul(
                                ps[:, j, :],
                                lhsT=xh_v[:, c, rsl],
                                rhs=ct_v[:, c, :],
                                start=(c == 0),
                                stop=False,
                            )
                        if USE_CCORR:
                            for jd in range(2):
                                nc.tensor.matmul(
                                    ps[:, j, :],
                                    lhsT=x8_v[:, jd, :, rsl],
                                    rhs=ctl_v[:, jd],
                                    start=False,
                                    stop=(jd == 1),
                                    perf_mode=mybir.MatmulPerfMode.DoubleRow,
                                )
                        else:
                            nc.tensor.matmul(
                                ps[:, j, :],
                                lhsT=bones_v[:],
                                rhs=brhs_v[:],
                                start=False,
                                stop=True,
                                perf_mode=mybir.MatmulPerfMode.DoubleRow,
                            )
                    negm = small.tile([P, pair], f32, tag="negm")
                    nc.vector.reduce_max(
                        out=negm[:], in_=ps[:], axis=mybir.AxisListType.X,
                        negate=True,
                    )
                    for j in range(pair):
                        nc.scalar.activation(
                            out=og[:, s0 + j, :],
                            in_=ps[:, j, :],
                            func=mybir.ActivationFunctionType.Exp,
                            bias=negm[:, j:j + 1],
                            scale=1.0,
                        )
                # per-group out DMA; the final group issues from the
                # (idle-at-tail) scalar queue.
                eng = nc.scalar if last_g else nc.sync
                eng.dma_start(out=out_v[:, t0:t0 + subtiles, :], in_=og[:])
                t0 += subtiles
    nc.finalize()
    return nc


def get_nc():
    if "nc" not in _CACHE:
        _CACHE["nc"] = _build_bass()
    return _CACHE["nc"]


def prep_inputs(y_pred: np.ndarray, mask: np.ndarray, centers: np.ndarray):
    """Host-side prep: valid-timestep slice, per-core transpose, fp16/fp8
    packing (one contiguous byte blob per DMA), bias splits, host-side
    softmax weight table."""
    x = np.ascontiguousarray(y_pred.reshape(B, T, D))
    masktime = np.asarray(mask).reshape(B, T, D)[0, :, 0]
    valid_idx = np.nonzero(masktime == 0)[0][:VALID_T]
    assert valid_idx.shape[0] == VALID_T
    if valid_idx[0] == 0 and valid_idx[-1] == VALID_T - 1:
        xv = x[:, :VALID_T]                    # [B, VALID_T, D]
    else:
        xv = x[:, valid_idx]

    centers64 = np.asarray(centers, dtype=np.float64)
    ct = (2.0 * centers64).T                                # [D, K]
    cth = ct.astype(F16)
    negc2 = -(centers64 ** 2).sum(axis=1)                   # [K]
    # 3-level e5m2 bias cascade, carried in ctl8 contraction rows 509-511
    b1 = negc2.astype(E5M2)
    r1 = negc2 - b1.astype(np.float64)
    b2 = r1.astype(E5M2)
    b3 = (r1 - b2.astype(np.float64)).astype(E5M2)

    # host-side per-center softmax weight: exact residual of the device bias
    lw = negc2 - (b1.astype(np.float64) + b2.astype(np.float64)
                  + b3.astype(np.float64))
    w_host = np.exp(lw - lw.max()).astype(np.float32)       # [K], ~1.0
    _CACHE["w_host"] = w_host

    parts = [
        np.ascontiguousarray(
            cth.reshape(DC, P, K).transpose(1, 0, 2)
        ).reshape(P, DC * K).view(np.uint8),
    ]
    if USE_CCORR:
        ctl = (ct - cth.astype(np.float64)).astype(E5M2)    # [D, K]
        ctl[509], ctl[510], ctl[511] = b1, b2, b3
        parts.append(np.ascontiguousarray(
            ctl.reshape(2, 2, P, K).transpose(2, 0, 1, 3)
        ).reshape(P, DC * K).view(np.uint8))
    else:
        # bias-only DoubleRow carrier: ones in lhsT rows 0-2 (slice 0),
        # 3-level e5m2 bias in the matching rhs rows
        bones = np.zeros((P, 2, P), dtype=E5M2)
        bones[0:3, 0, :] = 1.0
        brhs = np.zeros((P, 2, K), dtype=E5M2)
        brhs[0, 0], brhs[1, 0], brhs[2, 0] = b1, b2, b3
        parts.append(bones.reshape(P, 2 * P).view(np.uint8))
        parts.append(brhs.reshape(P, 2 * K).view(np.uint8))
    cp = np.ascontiguousarray(np.concatenate(parts, axis=1))
    assert cp.shape == (P, CONST_B)
    cp = cp.ravel()

    in_maps = []
    for core in range(N_CORES):
        xc = xv[core * B_PER_CORE:(core + 1) * B_PER_CORE].reshape(ROWS, D)
        xT = np.ascontiguousarray(xc.T)                     # [D, ROWS] f32
        xh = xT.astype(F16)
        xh_p = xh.reshape(DC, P, ROWS).transpose(1, 0, 2)   # [P, DC, ROWS]
        if USE_CCORR:
            x8 = xT.astype(E5M2)
            x8[509:512] = 1.0  # bias contraction rows (pair with ctl 509-511)
            x8_p = x8.reshape(2, 2, P, ROWS).transpose(2, 0, 1, 3)
        blocks = []
        r0 = 0
        for R in GROUPS:
            hb = np.ascontiguousarray(
                xh_p[:, :, r0:r0 + R]).reshape(P, DC * R).view(np.uint8)
            if USE_CCORR:
                lb = np.ascontiguousarray(
                    x8_p[:, :, :, r0:r0 + R]).reshape(P, DC * R).view(np.uint8)
                blocks.append(np.concatenate([hb, lb], axis=1).ravel())
            else:
                blocks.append(hb.copy().ravel())
            r0 += R
        xp_core = np.concatenate(blocks)
        assert xp_core.shape[0] == P * XB * ROWS
        in_maps.append({"xp": xp_core, "cp": cp})
    return in_maps


def kernel(y_pred: np.ndarray, mask: np.ndarray, centers: np.ndarray,
           **run_kwargs) -> np.ndarray:
    in_maps = prep_inputs(y_pred, mask, centers)
    nc = get_nc()
    last_err = None
    for _attempt in range(3):
        try:
            res = run_bass_kernel_spmd(nc, in_maps, core_ids=list(range(N_CORES)),
                                       **run_kwargs)
            break
        except Exception as e:  # transient NRT device errors — retry
            last_err = e
    else:
        raise last_err
    _CACHE["last_results"] = res
    e = np.concatenate(
        [np.asarray(r["out"]).reshape(B_PER_CORE, VALID_T, K)
         for r in res.results], axis=0
    ).astype(np.float32)
    ew = e * _CACHE["w_host"]
    out = ew / ew.sum(axis=-1, keepdims=True)
    return out.astype(np.float32, copy=False)


# revision 23
# speedup vs baseline: 1.1824x; 1.1824x over previous
"""Batch-assign-probability (VQ codebook softmax) kernel for 8 Trainium2 cores.

Math: for each valid row x (D=512), over K=256 centers c_k:
    softmax_k(-||x - c_k||^2) == softmax_k(2 x.c_k - ||c_k||^2)
(the ||x||^2 term is constant over k and cancels in softmax).

v2 design (vs the 3-pass bf16 baseline; 44.1us -> ~30.6us measured):
  - fp16 single-pass main matmul: x_fp16 @ (2c)_fp16^T, full PE rate. fp16's
    10+1 mantissa bits put the logit error at ~7e-3 rms; one extra
    correction pass kills the centers-side quantization error:
        corr = x_e5m2 @ ((2c) - fp16(2c))_e5m2
    run as 2 fp8 DoubleRow matmuls (256-contraction per inst, one weight
    load each). End-to-end max abs output err 9.58e-3 vs the 2e-2 gate
    (deterministic; HW matches the numpy simulation to ~1e-5).
  - The -||c||^2 bias rides the correction pass in 3 sacrificed contraction
    rows (x8 rows 509-511 := 1.0, ctl8 rows 509-511 := 3-level e5m2 bias
    split); the residual is compensated exactly on the host. No bias matmul.
  - exp(l - rowmax) on ACT writes fp16 directly; softmax NORMALIZATION
    (exp-bias residual weight + row-sum division) happens on the HOST,
    removing the accum-read + reciprocal + broadcast-multiply chain from
    the device. Scalar engine runs only the 16 exps (+1 tail out-DMA).
  - All x traffic for a row group rides ONE byte-packed DMA (fp16 bytes
    then fp8 bytes per partition), bitcast back into typed views on SBUF:
    3.1MB/core of x instead of 4.2MB, few descriptors on the sync ring
    (each DMA_DIRECT2D costs ~650ns of issue + the ring is FIFO, so
    fewer/bigger transfers win; out-DMAs ride behind the x stream).
  - Device per 128-row tile: 4 fp16 + 2 fp8-DR matmuls -> PSUM (the
    steady-state cost is 6 LDWEIGHTS ~= 670ns/tile; the PE issue itself is
    ~540ns), DVE reduce_max(negate) per pair, ACT exp -> fp16 og, per-group
    out-DMA (last group from the scalar queue to dodge the sync tail).
  - Set USE_CCORR=False for a ~0.8us faster / err 1.51e-2 variant that
    drops the correction pass and the x8 stream.
"""

import numpy as np
import ml_dtypes

import concourse.bacc as bacc
import concourse.tile as tile
from concourse import mybir
from concourse.bass_utils import run_bass_kernel_spmd

B, T, W, C, K = 16, 2048, 512, 1, 256
VALID_T = 1024
D = W * C                       # 512
N_CORES = 8
B_PER_CORE = B // N_CORES       # 2
ROWS = B_PER_CORE * VALID_T     # 2048 rows per core
P = 128
DC = D // P                     # 4 contraction chunks
GROUPS = [128, 256, 512, 512, 512, 128]   # rows per x/out DMA group
N_WARM_MM = 6                   # dummy matmuls to lift the PE HAM clock-gate
assert sum(GROUPS) == ROWS
assert all(g % P == 0 for g in GROUPS)

# USE_CCORR=True: a 2-inst fp8 DoubleRow pass corrects the fp16(2c)
# quantization error (max err ~9.6e-3) and carries the -||c||^2 bias in 3
# sacrificed contraction rows. USE_CCORR=False: skip the correction (max err
# ~1.50e-2, still under the 2e-2 gate), drop the x8 stream (2.1MB instead of
# 3.1MB of x per core) and carry the bias in a dedicated 1-inst DoubleRow
# matmul against tiny resident constants: 5 instead of 6 weight-load-bound
# matmuls per 128-row tile. Bias residual is compensated exactly by the
# host-side softmax weight either way.
USE_CCORR = True
XB = 12 if USE_CCORR else 8     # x bytes per row element-slot per partition
CONST_B = (2048 + 1024) if USE_CCORR else (2048 + 256 + 512)

F16 = np.float16
E5M2 = ml_dtypes.float8_e5m2

_CACHE: dict = {}


def _build_bass():
    f32 = mybir.dt.float32
    f16 = mybir.dt.float16
    f8 = mybir.dt.float8e5
    u8 = mybir.dt.uint8
    nc = bacc.Bacc()
    xp = nc.declare_dram_parameter("xp", [P * XB * ROWS], u8, isOutput=False)
    cp = nc.declare_dram_parameter("cp", [P * CONST_B], u8, isOutput=False)
    out = nc.declare_dram_parameter("out", [ROWS, K], f16, isOutput=True)
    out_v = out.rearrange("(t p) k -> p t k", p=P)       # [128, 16, 256]

    with tile.TileContext(nc) as tc:
        with (
            tc.tile_pool(name="singles", bufs=1) as singles,
            tc.tile_pool(name="xpool", bufs=1) as xpool,
            tc.tile_pool(name="opool", bufs=3) as opool,
            tc.tile_pool(name="small", bufs=8) as small,
            tc.tile_pool(name="psum", bufs=7, space="PSUM") as psum,
            tc.tile_pool(name="psum_warm", bufs=1, space="PSUM") as psum_warm,
        ):
            # constants first on the sync ring: they gate the first matmul
            # together with group 0. ct and ctl ride separate DMAs (deps are
            # byte-range granular) so the first fp16 matmuls gate on ct +
            # g0's xh bytes only (~0.39MB), not the full const+g0 0.59MB.
            csb = singles.tile([P, CONST_B], u8)
            c_ap = csb[:]
            cpv = cp.rearrange("(p b) -> p b", p=P)
            nc.sync.dma_start(out=c_ap[:], in_=cpv[:])
            ct_v = c_ap[:, :2 * DC * K].bitcast(f16).rearrange(
                "p (c k) -> p c k", c=DC)
            o = 2 * DC * K
            if not USE_CCORR:
                # bias carrier: lhsT ones (rows 0-2 of slice 0), rhs bias rows
                bones_v = c_ap[:, o:o + 2 * P].bitcast(f8).rearrange(
                    "p (i r) -> p i r", i=2)
                o += 2 * P
                brhs_v = c_ap[:, o:o + 2 * K].bitcast(f8).rearrange(
                    "p (i k) -> p i k", i=2)

            xgs = []
            xoff = 0
            for g, R in enumerate(GROUPS):
                n = P * XB * R
                xg = xpool.tile([P, XB * R], u8, tag=f"xg{g}")
                xsrc = xp[xoff:xoff + n].rearrange("(p b) -> p b", p=P)
                ap = xg[:]
                nc.sync.dma_start(out=ap[:], in_=xsrc)
                xoff += n
                xh_v = ap[:, :8 * R].bitcast(f16).rearrange(
                    "p (c r) -> p c r", c=DC)
                x8_v = None
                if USE_CCORR:
                    x8_v = ap[:, 8 * R:].bitcast(f8).rearrange(
                        "p (j i r) -> p j i r", j=2, i=2)
                xgs.append((xh_v, x8_v))
            ctl_v = None
            if USE_CCORR:
                ctl_v = c_ap[:, 2 * DC * K:].bitcast(f8).rearrange(
                    "p (j i k) -> p j i k", j=2, i=2)

            # PE warm-up: dummy matmuls keep the PE busy through the HAM
            # activity window while the first x DMA lands.
            warm_sb = singles.tile([P, 512], f16)
            nc.gpsimd.memset(warm_sb[:], 0.0)
            warm_ps = psum_warm.tile([P, 512], f32, tag="warm")
            for _ in range(N_WARM_MM):
                nc.tensor.matmul(
                    warm_ps[:], lhsT=warm_sb[:, :P], rhs=warm_sb[:],
                    start=True, stop=True,
                )

            t0 = 0  # running 128-row tile index
            for g, R in enumerate(GROUPS):
                xh_v, x8_v = xgs[g]
                subtiles = R // P
                og = opool.tile([P, subtiles, K], f16, tag="og")
                last_g = g == len(GROUPS) - 1
                for s0 in range(0, subtiles, 2):
                    pair = min(2, subtiles - s0)
                    ps = psum.tile([P, pair, K], f32, tag="ps")
                    for j in range(pair):
                        s = s0 + j
                        rsl = slice(s * P, (s + 1) * P)
                        for c in range(DC):
                            nc.tensor.matmul(
                                ps[:, j, :],
                                lhsT=xh_v[:, c, rsl],
                                rhs=ct_v[:, c, :],
                                start=(c == 0),
                                stop=False,
                            )
                        if USE_CCORR:
                            for jd in range(2):
                                nc.tensor.matmul(
                                    ps[:, j, :],
                                    lhsT=x8_v[:, jd, :, rsl],
                                    rhs=ctl_v[:, jd],
                                    start=False,
                                    stop=(jd == 1),
                                    perf_mode=mybir.MatmulPerfMode.DoubleRow,
                                )
                        else:
                            nc.tensor.matmul(
                                ps[:, j, :],
                                lhsT=bones_v[:],
                                rhs=brhs_v[:],
                                start=False,
                                stop=True,
                                perf_mode=mybir.MatmulPerfMode.DoubleRow,
                            )
                    negm = small.tile([P, pair], f32, tag="negm")
                    nc.vector.reduce_max(
                        out=negm[:], in_=ps[:], axis=mybir.AxisListType.X,
                        negate=True,
                    )
                    for j in range(pair):
                        nc.scalar.activation(
                            out=og[:, s0 + j, :],
                            in_=ps[:, j, :],
                            func=mybir.ActivationFunctionType.Exp,
                            bias=negm[:, j:j + 1],
                            scale=1.0,
                        )
                    # per-pair out DMA; the final group issues from the
                    # (idle-at-tail) scalar queue so the tail transfer is
                    # not stuck behind the sync ring
                    eng = nc.scalar if last_g else nc.sync
                    eng.dma_start(
                        out=out_v[:, t0 + s0:t0 + s0 + pair, :],
                        in_=og[:, s0:s0 + pair, :],
                    )
                t0 += subtiles
    nc.finalize()
    return nc


def get_nc():
    if "nc" not in _CACHE:
        _CACHE["nc"] = _build_bass()
    return _CACHE["nc"]


def prep_inputs(y_pred: np.ndarray, mask: np.ndarray, centers: np.ndarray):
    """Host-side prep: valid-timestep slice, per-core transpose, fp16/fp8
    packing (one contiguous byte blob per DMA), bias splits, host-side
    softmax weight table."""
    x = np.ascontiguousarray(y_pred.reshape(B, T, D))
    masktime = np.asarray(mask).reshape(B, T, D)[0, :, 0]
    valid_idx = np.nonzero(masktime == 0)[0][:VALID_T]
    assert valid_idx.shape[0] == VALID_T
    if valid_idx[0] == 0 and valid_idx[-1] == VALID_T - 1:
        xv = x[:, :VALID_T]                    # [B, VALID_T, D]
    else:
        xv = x[:, valid_idx]

    centers64 = np.asarray(centers, dtype=np.float64)
    ct = (2.0 * centers64).T                                # [D, K]
    cth = ct.astype(F16)
    negc2 = -(centers64 ** 2).sum(axis=1)                   # [K]
    # 3-level e5m2 bias cascade, carried in ctl8 contraction rows 509-511
    b1 = negc2.astype(E5M2)
    r1 = negc2 - b1.astype(np.float64)
    b2 = r1.astype(E5M2)
    b3 = (r1 - b2.astype(np.float64)).astype(E5M2)

    # host-side per-center softmax weight: exact residual of the device bias
    lw = negc2 - (b1.astype(np.float64) + b2.astype(np.float64)
                  + b3.astype(np.float64))
    w_host = np.exp(lw - lw.max()).astype(np.float32)       # [K], ~1.0
    _CACHE["w_host"] = w_host

    parts = [
        np.ascontiguousarray(
            cth.reshape(DC, P, K).transpose(1, 0, 2)
        ).reshape(P, DC * K).view(np.uint8),
    ]
    if USE_CCORR:
        ctl = (ct - cth.astype(np.float64)).astype(E5M2)    # [D, K]
        ctl[509], ctl[510], ctl[511] = b1, b2, b3
        parts.append(np.ascontiguousarray(
            ctl.reshape(2, 2, P, K).transpose(2, 0, 1, 3)
        ).reshape(P, DC * K).view(np.uint8))
    else:
        # bias-only DoubleRow carrier: ones in lhsT rows 0-2 (slice 0),
        # 3-level e5m2 bias in the matching rhs rows
        bones = np.zeros((P, 2, P), dtype=E5M2)
        bones[0:3, 0, :] = 1.0
        brhs = np.zeros((P, 2, K), dtype=E5M2)
        brhs[0, 0], brhs[1, 0], brhs[2, 0] = b1, b2, b3
        parts.append(bones.reshape(P, 2 * P).view(np.uint8))
        parts.append(brhs.reshape(P, 2 * K).view(np.uint8))
    cp = np.ascontiguousarray(np.concatenate(parts, axis=1))
    assert cp.shape == (P, CONST_B)
    cp = cp.ravel()

    in_maps = []
    for core in range(N_CORES):
        xc = xv[core * B_PER_CORE:(core + 1) * B_PER_CORE].reshape(ROWS, D)
        xT = np.ascontiguousarray(xc.T)                     # [D, ROWS] f32
        xh = xT.astype(F16)
        xh_p = xh.reshape(DC, P, ROWS).transpose(1, 0, 2)   # [P, DC, ROWS]
        if USE_CCORR:
            x8 = xT.astype(E5M2)
            x8[509:512] = 1.0  # bias contraction rows (pair with ctl 509-511)
            x8_p = x8.reshape(2, 2, P, ROWS).transpose(2, 0, 1, 3)
        blocks = []
        r0 = 0
        for R in GROUPS:
            hb = np.ascontiguousarray(
                xh_p[:, :, r0:r0 + R]).reshape(P, DC * R).view(np.uint8)
            if USE_CCORR:
                lb = np.ascontiguousarray(
                    x8_p[:, :, :, r0:r0 + R]).reshape(P, DC * R).view(np.uint8)
                blocks.append(np.concatenate([hb, lb], axis=1).ravel())
            else:
                blocks.append(hb.copy().ravel())
            r0 += R
        xp_core = np.concatenate(blocks)
        assert xp_core.shape[0] == P * XB * ROWS
        in_maps.append({"xp": xp_core, "cp": cp})
    return in_maps


def kernel(y_pred: np.ndarray, mask: np.ndarray, centers: np.ndarray,
           **run_kwargs) -> np.ndarray:
    in_maps = prep_inputs(y_pred, mask, centers)
    nc = get_nc()
    last_err = None
    for _attempt in range(3):
        try:
            res = run_bass_kernel_spmd(nc, in_maps, core_ids=list(range(N_CORES)),
                                       **run_kwargs)
            break
        except Exception as e:  # transient NRT device errors — retry
            last_err = e
    else:
        raise last_err
    _CACHE["last_results"] = res
    e = np.concatenate(
        [np.asarray(r["out"]).reshape(B_PER_CORE, VALID_T, K)
         for r in res.results], axis=0
    ).astype(np.float32)
    ew = e * _CACHE["w_host"]
    out = ew / ew.sum(axis=-1, keepdims=True)
    return out.astype(np.float32, copy=False)
